# revision 22
# baseline (speedup 1.0000x reference)
"""CfC (closed-form continuous-time RNN) kernel for Trainium2, 8 NeuronCores.

Data-parallel over batch: B=256 -> 32 samples/core. Per core the T=512
recurrence runs fully on-chip in feature-major layout ([feature, batch]
tiles) so every matmul uses the weights as the stationary operand and the
tiny 32-wide batch as the moving operand.

Numerics: matmul operands in fp16 (weights + activations), PSUM/fp32
accumulation, activations evaluated in fp32 on the ACT engine, outputs
fp32. Biases are accumulated into PSUM via small indicator matmuls. The
sigmoid gate is computed as 0.5 + 0.5*tanh(u/2) (0.5 folded into the
t-head weights) so one tanh instruction covers ff1|ff2|t in one PSUM
bank.

Scheduling note: the walrus codegen fits only ONE semaphore wait on most
engine instruction structs, so the emission is arranged so every
instruction needs at most one new wait: each engine observes the other
engines' clocks through a designated per-step instruction (h_bf cast on
ACT observes DVE; z/tanh observe PE; h-matmuls observe ACT; the split
h_bf cast second half carries the ACT self-chain for the shared z PSUM
bank), plus no-sync ordering pins that stop the scheduler from hoisting
dependency-free matmuls into positions where extra waits materialize.
"""

import numpy as np

B, T, D, H, U = 256, 512, 128, 256, 256
HCH = 16                  # blocks per hout chunk (4 chunks at T=512)
NCORES = 8
BL = B // NCORES          # 32 batch per core
TB = 8                    # time steps per block (one PSUM bank of z)
LECUN_A, LECUN_B = 1.7159, 0.666

_NC_CACHE = {}


def _build_nc(t_steps, mm_dt_name="float16"):
    import concourse.bass as bass
    import concourse.mybir as mybir
    import concourse.tile as tile
    from concourse.tile import add_dep_helper

    def _raw(r):
        return getattr(r, "ins", r)

    f32 = mybir.dt.float32
    f16 = getattr(mybir.dt, mm_dt_name)
    nb = t_steps // TB

    nc = bass.Bass()

    assert nb % HCH == 0 or nb <= HCH
    nch = max(1, nb // HCH)               # hout chunks (<=4)
    cb = nb // nch                        # blocks per hout chunk
    xin_d = nc.declare_dram_parameter("xin", [128, nb * TB * BL], f16, isOutput=False)
    wbig_d = nc.declare_dram_parameter("wbig", [128, 2304], f16, isOutput=False)
    wsml_d = nc.declare_dram_parameter("wsml", [6, 960], f16, isOutput=False)
    hout_d = nc.declare_dram_parameter("hout", [nch, 128, cb * TB * 2 * BL], f32,
                                       isOutput=True)

    AOP = mybir.AluOpType
    ACT = mybir.ActivationFunctionType

    with tile.TileContext(nc) as tc:
        with (
            tc.tile_pool(name="wpool", bufs=1) as wpool,
            tc.tile_pool(name="xpool", bufs=1) as xpool,
            tc.tile_pool(name="zpool", bufs=3) as zpool,
            tc.tile_pool(name="ffpool", bufs=3) as ffpool,
            tc.tile_pool(name="dpool", bufs=2) as dpool,
            tc.tile_pool(name="cpool", bufs=2) as cpool,
            tc.tile_pool(name="epool", bufs=2) as epool,
            tc.tile_pool(name="hpool", bufs=3) as hpool,
            tc.tile_pool(name="histpool", bufs=2) as histpool,
            tc.tile_pool(name="zppool", bufs=2, space="PSUM") as zppool,
            tc.tile_pool(name="fppool", bufs=3, space="PSUM") as fppool,
            tc.tile_pool(name="warmpool", bufs=1, space="PSUM") as warmpool,
        ):
            dma_insts = []
            # --- constants (two DMAs; one per SBUF tile) ---
            wbig = wpool.tile([128, 2304], f16)
            dma_insts.append(nc.sync.dma_start(wbig[:], wbig_d[:]))
            wsml = wpool.tile([6, 960], f16)
            dma_insts.append(nc.sync.dma_start(wsml[:], wsml_d[:]))

            # layout of wbig cols: wx [0:256), wh [256:768), wf [768:2304)
            wx_sb = wbig[:, 0:256]
            wh_sb = wbig[:, 256:768]
            wf_sb = wbig[:, 768:2304]
            # layout of wsml cols: bz rows0-1 [0:128), bf rows0-5 [128:320),
            # indz rows0-1 [320:832), indf rows0-5 [832:960? 192 cols]
            bz_sb = wsml[0:2, 0:128]
            bf_sb = wsml[0:6, 128:256]
            indz = wsml[0:2, 256:768]
            indf = wsml[0:6, 768:960]

            # warm-up matmul: reads only wbig, so the very first PE
            # instruction carries the single big-const DMA-lane wait and all
            # later matmuls find that lane already observed.
            warm = warmpool.tile([128, 8], f32)
            nc.tensor.matmul(warm[:, 0:1], wbig[:, 0:128], wbig[:, 0:1],
                             start=True, stop=True, skip_group_check=True)
            # same for wsml (different DMA lane): tiny matmul reading wsml only
            nc.tensor.matmul(warm[0:6, 2:3], wsml[:, 0:6], wsml[:, 0:1],
                             start=True, stop=True, skip_group_check=True)

            # whole input staged in SBUF with ONE DMA: with <=8 total DMAs
            # in the program, no DMAHW lane is ever reused, so no DMA needs
            # an own-lane FIFO wait on top of its data wait (1-wait limit)
            x_all = xpool.tile([128, nb * TB * BL], f16)
            dma_insts.append(nc.sync.dma_start(x_all[:], xin_d[:]))

            # initial hidden state (fp16 copy used as matmul rhs); memset on
            # DVE so later WAW deps on this pool slot merge into DVE waits
            h_bf = hpool.tile([128, 2 * BL], f16, tag="hbf")
            nc.vector.memset(h_bf[:], 0.0)

            last_act_z = None
            h_hist = None
            for tb in range(nb):
                x_sb = x_all[:, tb * TB * BL : (tb + 1) * TB * BL]

                # z pre-activations for all TB steps of this block:
                # zp[:, m*256 + s*32 + j] = (Wb.T @ [x;h])[m*128+p] + bb
                zp = zppool.tile([128, 2 * TB * BL], f32)
                mm_x0 = nc.tensor.matmul(zp[:, 0:256], wx_sb[:, 0:128], x_sb,
                                         start=True, stop=False, skip_group_check=True)
                nc.tensor.matmul(zp[:, 256:512], wx_sb[:, 128:256], x_sb,
                                 start=False, stop=False, skip_group_check=True)
                nc.tensor.matmul(zp[:], bz_sb, indz, start=False, stop=False,
                                 skip_group_check=True)
                if last_act_z is not None:
                    # don't hoist block matmuls ahead of the previous block's
                    # tail (keeps their PSUM-WAR deps transitively covered)
                    add_dep_helper(_raw(mm_x0), _raw(last_act_z), sync=False,
                                   reason="block mm after prev z act")

                zp3 = zp[:].rearrange("p (m t) -> p m t", m=2)
                if tb % cb == 0:
                    h_hist = histpool.tile([128, cb * TB * 2 * BL], f32)
                hoff = (tb % cb) * TB * 2 * BL

                for s in range(TB):
                    sl = slice(s * BL, (s + 1) * BL)
                    # backbone: accumulate Wh.T @ h into this step's z slice
                    for k in range(2):
                        for m in range(2):
                            nc.tensor.matmul(
                                zp[:, m * 256 + s * BL : m * 256 + (s + 1) * BL],
                                wh_sb[:, (k * 2 + m) * 128 : (k * 2 + m + 1) * 128],
                                h_bf[:, k * BL : (k + 1) * BL],
                                start=False, stop=False,
                                skip_group_check=True,
                            )
                    # z = tanh(0.666 * pre); the 1.7159 is folded into wf
                    z_sb = zpool.tile([128, 2 * BL], f16)
                    act_z = nc.scalar.activation(
                        z_sb[:].rearrange("p (m j) -> p m j", m=2),
                        zp3[:, :, sl], ACT.Tanh, scale=LECUN_B,
                    )
                    last_act_z = act_z
                    # ff1 | ff2 | t-head pre-activations + biases, one bank
                    fpm = fppool.tile([128, 6 * BL], f32)
                    mm_bias = nc.tensor.matmul(fpm[:], bf_sb, indf, start=True,
                                               stop=False, skip_group_check=True)
                    add_dep_helper(_raw(mm_bias), _raw(act_z), sync=False,
                                   reason="bias mm after z act")
                    for head in range(3):
                        for k in range(2):
                            for m in range(2):
                                col = (head * 2 + m) * BL
                                nc.tensor.matmul(
                                    fpm[:, col : col + BL],
                                    wf_sb[:, ((head * 2 + k) * 2 + m) * 128 :
                                          ((head * 2 + k) * 2 + m + 1) * 128],
                                    z_sb[:, k * BL : (k + 1) * BL],
                                    start=False, stop=False,
                                    skip_group_check=True,
                                )
                    # tanh over all three heads; t-head holds u/2 so that
                    # sigma = 0.5 + 0.5*tau
                    fft = ffpool.tile([128, 6 * BL], f32)
                    nc.scalar.activation(fft[:], fpm[:], ACT.Tanh)

                    # h = ff1 + sigma*(ff2-ff1) = ff1 + 0.5*(1+tau)*(ff2-ff1)
                    dsb = dpool.tile([128, 2 * BL], f32)
                    nc.vector.tensor_tensor(dsb[:], fft[:, 2 * BL : 4 * BL],
                                            fft[:, 0 : 2 * BL], AOP.subtract)
                    csb = cpool.tile([128, 2 * BL], f32)
                    nc.vector.tensor_tensor(csb[:], fft[:, 4 * BL : 6 * BL],
                                            dsb[:], AOP.mult)
                    esb = epool.tile([128, 2 * BL], f32)
                    nc.vector.tensor_tensor(esb[:], dsb[:], csb[:], AOP.add)
                    hs = h_hist[:, hoff + s * 2 * BL : hoff + (s + 1) * 2 * BL]
                    nc.vector.scalar_tensor_tensor(hs, esb[:], 0.5,
                                                   fft[:, 0 : 2 * BL],
                                                   AOP.mult, AOP.add)

                    # h_bf cast on ACT, split in two: first half observes the
                    # DVE clock (covers ACT-side WAR deps), second half chains
                    # the ACT self-clock past act_z (shared z-bank serializer)
                    h_bf = hpool.tile([128, 2 * BL], f16, tag="hbf")
                    nc.scalar.copy(h_bf[:, 0:BL], hs[:, 0:BL])
                    cp_b = nc.scalar.copy(h_bf[:, BL : 2 * BL], hs[:, BL : 2 * BL])
                    add_dep_helper(_raw(cp_b), _raw(act_z), sync=True,
                                   reason="carry act self-clock past z")

                if tb % cb == cb - 1:
                    dma_insts.append(nc.sync.dma_start(hout_d[tb // cb], h_hist[:]))
                    # tiny DVE write touching h_hist after its DMA read:
                    # absorbs the DMA-lane wait into the DVE clock so the next
                    # reuse of this hist slot needs only a DVE-self wait
                    last_touch = nc.vector.tensor_copy(h_hist[0:1, 0:1],
                                                       h_hist[0:1, 1:2])

            # --- end-of-program semaphore consumption (see module docstring):
            # an SP nop chain waits each proc's final tick so the single tail
            # drain instruction needs no waits of its own.
            mm_end = nc.tensor.matmul(warm[0:64, 3:4], h_bf[:, 0:64],
                                      h_bf[:, 0:1], start=True, stop=True,
                                      skip_group_check=True)
            for tgt in [cp_b, mm_end, last_touch] + dma_insts:
                nsp = nc.sync.nop()
                add_dep_helper(_raw(nsp), _raw(tgt), sync=True,
                               reason="tail sem consumption")

    nc.freeze()
    return nc


def _pack_weights(inputs, np_dt):
    Wb = np.asarray(inputs["Wb"], np.float32)
    wx = Wb[:128]                                                  # [128, 256]
    wh = Wb[128:].reshape(2, 128, 2, 128).transpose(1, 0, 2, 3).reshape(128, 512)
    heads = [
        LECUN_A * np.asarray(inputs["Wff1"], np.float32),
        LECUN_A * np.asarray(inputs["Wff2"], np.float32),
        0.5 * LECUN_A * (np.asarray(inputs["Wta"], np.float32)
                         + np.asarray(inputs["Wtb"], np.float32)),
    ]
    wf = np.stack(
        [w.reshape(2, 128, 2, 128).transpose(1, 0, 2, 3) for w in heads], axis=1
    ).reshape(128, 1536)
    wbig = np.concatenate([wx, wh, wf], axis=1)                    # [128, 2304]

    bz = np.asarray(inputs["bb"], np.float32).reshape(2, 128)
    bff1 = np.asarray(inputs["bff1"], np.float32)
    bff2 = np.asarray(inputs["bff2"], np.float32)
    bt = 0.5 * (np.asarray(inputs["bta"], np.float32)
                + np.asarray(inputs["btb"], np.float32))
    bf = np.stack([bff1[:128], bff1[128:], bff2[:128], bff2[128:],
                   bt[:128], bt[128:]])                            # [6, 128]
    indz = np.zeros((2, 512), np.float32)
    for m in range(2):
        indz[m, m * 256 : (m + 1) * 256] = 1.0
    indf = np.zeros((6, 192), np.float32)
    for r in range(6):
        indf[r, r * 32 : (r + 1) * 32] = 1.0
    wsml = np.zeros((6, 960), np.float32)
    wsml[0:2, 0:128] = bz
    wsml[0:6, 128:256] = bf
    wsml[0:2, 256:768] = indz
    wsml[0:6, 768:960] = indf
    return dict(wbig=np.ascontiguousarray(wbig).astype(np_dt),
                wsml=np.ascontiguousarray(wsml).astype(np_dt))


def _pack_x_core(x_core, t_steps, np_dt):
    # x_core [BL, T, D] -> xin [128, nb*TB*BL]; xin[d, (tb*TB+s)*BL+j] = x[j, tb*TB+s, d]
    xr = x_core.transpose(2, 1, 0).reshape(D, t_steps * BL)
    return np.ascontiguousarray(xr).astype(np_dt)


def _unpack_hout(hout, t_steps):
    # hout [nch, 128, cb*TB*2*BL] f32 -> readout_core [BL, T, H]
    nb = t_steps // TB
    nch = max(1, nb // HCH)
    r = hout.reshape(nch, 128, (nb // nch) * TB, 2, BL)
    r = r.transpose(4, 0, 2, 3, 1).reshape(BL, t_steps, H)
    return np.ascontiguousarray(r)


def run(inputs, t_steps=T, mm_dt_name="float16", trace=False):
    import ml_dtypes
    from concourse.bass_utils import run_bass_kernel_spmd

    np_dt = {"float16": np.float16, "bfloat16": ml_dtypes.bfloat16}[mm_dt_name]
    key = (t_steps, mm_dt_name)
    if key not in _NC_CACHE:
        _NC_CACHE[key] = _build_nc(t_steps, mm_dt_name)
    nc = _NC_CACHE[key]

    w = _pack_weights(inputs, np_dt)
    x = np.asarray(inputs["x"], np.float32)
    in_maps = []
    for c in range(NCORES):
        m = dict(w)
        m["xin"] = _pack_x_core(x[c * BL : (c + 1) * BL, :t_steps], t_steps, np_dt)
        in_maps.append(m)

    res = run_bass_kernel_spmd(nc, in_maps, core_ids=list(range(NCORES)), trace=trace)
    readout = np.empty((B, t_steps, H), np.float32)
    for c in range(NCORES):
        readout[c * BL : (c + 1) * BL] = _unpack_hout(res.results[c]["hout"], t_steps)
    hT = np.ascontiguousarray(readout[:, -1, :])
    return (readout, hT), res


def kernel(**inputs):
    (readout, hT), _ = run(inputs)
    return readout, hT


# revision 31
# speedup vs baseline: 1.2874x; 1.2874x over previous
"""CfC (closed-form continuous-time RNN) kernel for Trainium2, 8 NeuronCores.

Data-parallel over batch: B=256 -> 32 samples/core. Per core the T=512
recurrence runs fully on-chip in feature-major layout ([feature, batch]
tiles) so every matmul uses the weights as the stationary operand and the
tiny 32-wide batch as the moving operand.

Numerics: matmul operands in fp16 (weights + activations), PSUM/fp32
accumulation, activations evaluated in fp32 on the ACT engine, outputs
fp32. Biases are accumulated into PSUM via small indicator matmuls. The
sigmoid gate is computed as 0.5 + 0.5*tanh(u/2) (0.5 folded into the
t-head weights) so one tanh instruction covers ff1|ff2|t in one PSUM
bank.

Scheduling note: the walrus codegen fits only ONE semaphore wait on most
engine instruction structs, so the emission is arranged so every
instruction needs at most one new wait: each engine observes the other
engines' clocks through a designated per-step instruction (h_bf cast on
ACT observes DVE; z/tanh observe PE; h-matmuls observe ACT; the split
h_bf cast second half carries the ACT self-chain for the shared z PSUM
bank), plus no-sync ordering pins that stop the scheduler from hoisting
dependency-free matmuls into positions where extra waits materialize.
"""

import numpy as np

B, T, D, H, U = 256, 512, 128, 256, 256
HCH = 16                  # blocks per hout chunk (4 chunks at T=512)
NCORES = 8
BL = B // NCORES          # 32 batch per core
TB = 8                    # time steps per block (one PSUM bank of z)
LECUN_A, LECUN_B = 1.7159, 0.666

_NC_CACHE = {}


def _build_nc(t_steps, mm_dt_name="float16"):
    import concourse.bass as bass
    import concourse.mybir as mybir
    import concourse.tile as tile
    from concourse.tile import add_dep_helper

    def _raw(r):
        return getattr(r, "ins", r)

    f32 = mybir.dt.float32
    f16 = getattr(mybir.dt, mm_dt_name)
    nb = t_steps // TB

    nc = bass.Bass()

    assert nb % HCH == 0 or nb <= HCH
    nch = max(1, nb // HCH)               # hout chunks (<=4)
    cb = nb // nch                        # blocks per hout chunk
    xin_d = nc.declare_dram_parameter("xin", [128, nb * TB * BL], f16, isOutput=False)
    wbig_d = nc.declare_dram_parameter("wbig", [128, 2816], f16, isOutput=False)
    wsml_d = nc.declare_dram_parameter("wsml", [6, 960], f16, isOutput=False)
    hout_d = nc.declare_dram_parameter("hout", [nch, 128, cb * TB * 2 * BL], f32,
                                       isOutput=True)

    AOP = mybir.AluOpType
    ACT = mybir.ActivationFunctionType

    with tile.TileContext(nc) as tc:
        with (
            tc.tile_pool(name="wpool", bufs=1) as wpool,
            tc.tile_pool(name="xpool", bufs=1) as xpool,
            tc.tile_pool(name="zpool", bufs=3) as zpool,
            tc.tile_pool(name="ffpool", bufs=3) as ffpool,
            tc.tile_pool(name="dpool", bufs=2) as dpool,
            tc.tile_pool(name="cpool", bufs=2) as cpool,
            tc.tile_pool(name="epool", bufs=2) as epool,
            tc.tile_pool(name="obspool", bufs=2) as obspool,
            tc.tile_pool(name="dobspool", bufs=2) as dobspool,
            tc.tile_pool(name="histpool", bufs=2) as histpool,
            tc.tile_pool(name="zppool", bufs=2, space="PSUM") as zppool,
            tc.tile_pool(name="fppool", bufs=3, space="PSUM") as fppool,
            tc.tile_pool(name="warmpool", bufs=1, space="PSUM") as warmpool,
        ):
            dma_insts = []
            # --- constants (two DMAs; one per SBUF tile) ---
            wbig = wpool.tile([128, 2816], f16)
            dma_insts.append(nc.sync.dma_start(wbig[:], wbig_d[:]))
            wsml = wpool.tile([6, 960], f16)
            dma_insts.append(nc.sync.dma_start(wsml[:], wsml_d[:]))

            # wbig cols: wx [0:256) wh [256:768) wf [768:2304) wh2 [2304:2816)
            wx_sb = wbig[:, 0:256]
            wh_sb = wbig[:, 256:768]
            wf_sb = wbig[:, 768:2304]
            wh2_sb = wbig[:, 2304:2816]
            # layout of wsml cols: bz rows0-1 [0:128), bf rows0-5 [128:320),
            # indz rows0-1 [320:832), indf rows0-5 [832:960? 192 cols]
            bz_sb = wsml[0:2, 0:128]
            bf_sb = wsml[0:6, 128:256]
            indz = wsml[0:2, 256:768]
            indf = wsml[0:6, 768:960]

            # warm-up matmul: reads only wbig, so the very first PE
            # instruction carries the single big-const DMA-lane wait and all
            # later matmuls find that lane already observed.
            warm = warmpool.tile([128, 8], f32)
            nc.tensor.matmul(warm[:, 0:1], wbig[:, 0:128], wbig[:, 0:1],
                             start=True, stop=True, skip_group_check=True)
            # same for wsml (different DMA lane): tiny matmul reading wsml only
            nc.tensor.matmul(warm[0:6, 2:3], wsml[:, 0:6], wsml[:, 0:1],
                             start=True, stop=True, skip_group_check=True)

            # whole input staged in SBUF with ONE DMA: with <=8 total DMAs
            # in the program, no DMAHW lane is ever reused, so no DMA needs
            # an own-lane FIFO wait on top of its data wait (1-wait limit)
            x_all = xpool.tile([128, nb * TB * BL], f16)
            dma_insts.append(nc.sync.dma_start(x_all[:], xin_d[:]))

            last_act_z = None
            h_hist = None
            fft_prev = None
            e_prev = None
            obsA_prev = cpb_prev = None
            for tb in range(nb):
                x_sb = x_all[:, tb * TB * BL : (tb + 1) * TB * BL]

                # z pre-activations for all TB steps of this block:
                # zp[:, m*256 + s*32 + j] = (Wb.T @ [x;h])[m*128+p] + bb
                zp = zppool.tile([128, 2 * TB * BL], f32)
                mm_x0 = nc.tensor.matmul(zp[:, 0:256], wx_sb[:, 0:128], x_sb,
                                         start=True, stop=False, skip_group_check=True)
                nc.tensor.matmul(zp[:, 256:512], wx_sb[:, 128:256], x_sb,
                                 start=False, stop=False, skip_group_check=True)
                nc.tensor.matmul(zp[:], bz_sb, indz, start=False, stop=False,
                                 skip_group_check=True)
                if last_act_z is not None:
                    # don't hoist block matmuls ahead of the previous block's
                    # tail (keeps their PSUM-WAR deps transitively covered)
                    add_dep_helper(_raw(mm_x0), _raw(last_act_z), sync=False,
                                   reason="block mm after prev z act")

                zp3 = zp[:].rearrange("p (m t) -> p m t", m=2)
                if tb % cb == 0:
                    h_hist = histpool.tile([128, cb * TB * 2 * BL], f32)
                hoff = (tb % cb) * TB * 2 * BL

                for s in range(TB):
                    sl = slice(s * BL, (s + 1) * BL)
                    # backbone h-part, split: h = ff1 + 0.5*e, so
                    # Wh@h = Wh@ff1 (ready right after the previous tanh,
                    # fills PE while DVE runs) + (0.5*Wh)@e (critical path)
                    if fft_prev is not None:
                        for k in range(2):
                            for m in range(2):
                                nc.tensor.matmul(
                                    zp[:, m * 256 + s * BL : m * 256 + (s + 1) * BL],
                                    wh_sb[:, (k * 2 + m) * 128 : (k * 2 + m + 1) * 128],
                                    fft_prev[:, k * BL : (k + 1) * BL],
                                    start=False, stop=False,
                                    skip_group_check=True,
                                )
                        mm_p2 = None
                        for k in range(2):
                            for m in range(2):
                                mm_p2 = nc.tensor.matmul(
                                    zp[:, m * 256 + s * BL : m * 256 + (s + 1) * BL],
                                    wh2_sb[:, (k * 2 + m) * 128 : (k * 2 + m + 1) * 128],
                                    e_prev[:, k * BL : (k + 1) * BL],
                                    start=False, stop=False,
                                    skip_group_check=True,
                                )
                    obs_t = obspool.tile([128, 4], f32)
                    # z = tanh(0.666 * pre); the 1.7159 is folded into wf
                    z_sb = zpool.tile([128, 2 * BL], f16)
                    act_z = nc.scalar.activation(
                        z_sb[:].rearrange("p (m j) -> p m j", m=2),
                        zp3[:, :, sl], ACT.Tanh, scale=LECUN_B,
                    )
                    if obsA_prev is not None:
                        add_dep_helper(_raw(act_z), _raw(obsA_prev), sync=False,
                                       reason="keep tiny act ops in their step")
                        add_dep_helper(_raw(act_z), _raw(cpb_prev), sync=False,
                                       reason="keep tiny act ops in their step")
                    last_act_z = act_z
                    # ff1 | ff2 | t-head pre-activations + biases, one bank
                    fpm = fppool.tile([128, 6 * BL], f32, tag="fpm")
                    mm_bias = nc.tensor.matmul(fpm[:], bf_sb, indf, start=True,
                                               stop=False, skip_group_check=True)
                    add_dep_helper(_raw(mm_bias), _raw(act_z), sync=False,
                                   reason="bias mm after z act")
                    for head in range(3):
                        for k in range(2):
                            for m in range(2):
                                col = (head * 2 + m) * BL
                                mm_ff = nc.tensor.matmul(
                                    fpm[:, col : col + BL],
                                    wf_sb[:, ((head * 2 + k) * 2 + m) * 128 :
                                          ((head * 2 + k) * 2 + m + 1) * 128],
                                    z_sb[:, k * BL : (k + 1) * BL],
                                    start=False, stop=False,
                                    skip_group_check=True,
                                )
                    # DVE needs a fresh PE-clock observation each step so its
                    # buffer-reuse WARs (e/d/c pools vs part2 matmul reads)
                    # stay covered. PE can't write SBUF and the live PSUM
                    # banks are ACT-read (same-bank serialization), so bounce
                    # through a tiny matmul into the otherwise-unused warm
                    # bank, then a tiny DVE read of it.
                    mm_obs = nc.tensor.matmul(warm[0:1, 4:5], z_sb[:, 0:1],
                                              z_sb[:, 0:1], start=True, stop=True,
                                              skip_group_check=True)
                    add_dep_helper(_raw(mm_obs), _raw(mm_ff), sync=False,
                                   reason="mm_obs after step matmuls")
                    dobs = dobspool.tile([1, 2], f32)
                    dve_obs = nc.vector.tensor_copy(dobs[0:1, 0:1], warm[0:1, 4:5])
                    # tanh over all three heads (fp16 out, feeds matmuls
                    # and 2x-mode DVE); t-head holds u/2: sigma = 0.5+0.5*tau
                    fft = ffpool.tile([128, 6 * BL], f16)
                    nc.scalar.activation(fft[:], fpm[:], ACT.Tanh)

                    # h = ff1 + sigma*(ff2-ff1) = ff1 + 0.5*(1+tau)*(ff2-ff1)
                    # e = (1+tau)*(ff2-ff1); recurrence uses ff1,e via matmuls
                    dsb = dpool.tile([128, 2 * BL], f16)
                    tt_sub = nc.vector.tensor_tensor(dsb[:], fft[:, 2 * BL : 4 * BL],
                                            fft[:, 0 : 2 * BL], AOP.subtract)
                    add_dep_helper(_raw(tt_sub), _raw(dve_obs), sync=False,
                                   reason="dve obs before combine")
                    csb = cpool.tile([128, 2 * BL], f16)
                    nc.vector.tensor_tensor(csb[:], fft[:, 4 * BL : 6 * BL],
                                            dsb[:], AOP.mult)
                    esb = epool.tile([128, 2 * BL], f16)
                    nc.vector.tensor_tensor(esb[:], dsb[:], csb[:], AOP.add)
                    hs = h_hist[:, hoff + s * 2 * BL : hoff + (s + 1) * 2 * BL]
                    stt = nc.vector.scalar_tensor_tensor(hs, esb[:], 0.5,
                                                         fft[:, 0 : 2 * BL],
                                                         AOP.mult, AOP.add)

                    # tiny ACT ops: first observes the DVE clock (covers ACT
                    # WARs on fft/z_sb bufs), second reads z_sb so its natural
                    # ACT-self wait carries the shared-z-bank serialization
                    # chain forward for the next step's z activation
                    obsA = nc.scalar.copy(obs_t[0:1, 0:1],
                                          h_hist[0:1, hoff + s * 2 * BL :
                                                 hoff + s * 2 * BL + 1])
                    cp_b = nc.scalar.copy(obs_t[0:1, 1:2],
                                          z_sb[0:1, 0:1])
                    obsA_prev, cpb_prev = obsA, cp_b
                    fft_prev = fft
                    e_prev = esb

                if tb % cb == cb - 1:
                    dma_insts.append(nc.sync.dma_start(hout_d[tb // cb], h_hist[:]))
                    # tiny DVE write touching h_hist after its DMA read:
                    # absorbs the DMA-lane wait into the DVE clock so the next
                    # reuse of this hist slot needs only a DVE-self wait
                    last_touch = nc.vector.tensor_copy(h_hist[0:1, 0:1],
                                                       h_hist[0:1, 1:2])

            # --- end-of-program semaphore consumption (see module docstring):
            # an SP nop chain waits each proc's final tick so the single tail
            # drain instruction needs no waits of its own.
            for tgt in [cp_b, obsA, mm_obs, stt, last_touch] + dma_insts:
                nsp = nc.sync.nop()
                add_dep_helper(_raw(nsp), _raw(tgt), sync=True,
                               reason="tail sem consumption")

    nc.freeze()
    return nc


def _pack_weights(inputs, np_dt):
    Wb = np.asarray(inputs["Wb"], np.float32)
    wx = Wb[:128]                                                  # [128, 256]
    wh = Wb[128:].reshape(2, 128, 2, 128).transpose(1, 0, 2, 3).reshape(128, 512)
    heads = [
        LECUN_A * np.asarray(inputs["Wff1"], np.float32),
        LECUN_A * np.asarray(inputs["Wff2"], np.float32),
        0.5 * LECUN_A * (np.asarray(inputs["Wta"], np.float32)
                         + np.asarray(inputs["Wtb"], np.float32)),
    ]
    wf = np.stack(
        [w.reshape(2, 128, 2, 128).transpose(1, 0, 2, 3) for w in heads], axis=1
    ).reshape(128, 1536)
    wbig = np.concatenate([wx, wh, wf, 0.5 * wh], axis=1)          # [128, 2816]

    bz = np.asarray(inputs["bb"], np.float32).reshape(2, 128)
    bff1 = np.asarray(inputs["bff1"], np.float32)
    bff2 = np.asarray(inputs["bff2"], np.float32)
    bt = 0.5 * (np.asarray(inputs["bta"], np.float32)
                + np.asarray(inputs["btb"], np.float32))
    bf = np.stack([bff1[:128], bff1[128:], bff2[:128], bff2[128:],
                   bt[:128], bt[128:]])                            # [6, 128]
    indz = np.zeros((2, 512), np.float32)
    for m in range(2):
        indz[m, m * 256 : (m + 1) * 256] = 1.0
    indf = np.zeros((6, 192), np.float32)
    for r in range(6):
        indf[r, r * 32 : (r + 1) * 32] = 1.0
    wsml = np.zeros((6, 960), np.float32)
    wsml[0:2, 0:128] = bz
    wsml[0:6, 128:256] = bf
    wsml[0:2, 256:768] = indz
    wsml[0:6, 768:960] = indf
    return dict(wbig=np.ascontiguousarray(wbig).astype(np_dt),
                wsml=np.ascontiguousarray(wsml).astype(np_dt))


def _pack_x_core(x_core, t_steps, np_dt):
    # x_core [BL, T, D] -> xin [128, nb*TB*BL]; xin[d, (tb*TB+s)*BL+j] = x[j, tb*TB+s, d]
    xr = x_core.transpose(2, 1, 0).reshape(D, t_steps * BL)
    return np.ascontiguousarray(xr).astype(np_dt)


def _unpack_hout(hout, t_steps):
    # hout [nch, 128, cb*TB*2*BL] f32 -> readout_core [BL, T, H]
    nb = t_steps // TB
    nch = max(1, nb // HCH)
    r = hout.reshape(nch, 128, (nb // nch) * TB, 2, BL)
    r = r.transpose(4, 0, 2, 3, 1).reshape(BL, t_steps, H)
    return np.ascontiguousarray(r)


def run(inputs, t_steps=T, mm_dt_name="float16", trace=False):
    import ml_dtypes
    from concourse.bass_utils import run_bass_kernel_spmd

    np_dt = {"float16": np.float16, "bfloat16": ml_dtypes.bfloat16}[mm_dt_name]
    key = (t_steps, mm_dt_name)
    if key not in _NC_CACHE:
        _NC_CACHE[key] = _build_nc(t_steps, mm_dt_name)
    nc = _NC_CACHE[key]

    w = _pack_weights(inputs, np_dt)
    x = np.asarray(inputs["x"], np.float32)
    in_maps = []
    for c in range(NCORES):
        m = dict(w)
        m["xin"] = _pack_x_core(x[c * BL : (c + 1) * BL, :t_steps], t_steps, np_dt)
        in_maps.append(m)

    res = run_bass_kernel_spmd(nc, in_maps, core_ids=list(range(NCORES)), trace=trace)
    readout = np.empty((B, t_steps, H), np.float32)
    for c in range(NCORES):
        readout[c * BL : (c + 1) * BL] = _unpack_hout(res.results[c]["hout"], t_steps)
    hT = np.ascontiguousarray(readout[:, -1, :])
    return (readout, hT), res


def kernel(**inputs):
    (readout, hT), _ = run(inputs)
    return readout, hT
